# revision 14
# baseline (speedup 1.0000x reference)
"""MultiHeadAttention Trainium2 kernel (8 NeuronCores, SPMD).

Sharding: data-parallel over batch (B=2), tensor-parallel over heads
(16 heads -> 4 per core).  Core c handles batch b=c//4, head group
g=c%4 (heads 4g..4g+3).  Wq/Wk/Wv are split column-wise, Wo row-wise;
the per-core Wo partial outputs are summed on the host (replaces the
all-reduce).

Device dataflow per core (bf16 matmuls, f32 PSUM accumulation):
  qT = Wq_g^T x^T   [256, 2048]   (heads on partitions, dh=64 each)
  kT = Wk_g^T x^T   [256, 2048]
  v  = x Wv_g       [2048, 256] stored interleaved with a ones column
                    per head: vaug[st] = [vA|1|vB|1|vC|1|vD|1]
  per (s_q chunk of 512, head pair):
    logitsT[s_k, s_q] = kT^T qT / 8       (two heads packed in PE row
                                           groups, K=64 each)
    p = exp(logitsT)  on ScalarE, scale=1/8 fused, bf16 out
    accT[65, s_q] += vaug_h^T p           (row 64 = softmax denominator)
    outcatT[h] = accT[0:64] * bcast(1/accT[64])   (deferred softmax norm)
  partial = outcatT^T Wo_g  -> DRAM f32
"""

import sys

import numpy as np

sys.path.insert(0, "/opt/trn_rl_repo")

import ml_dtypes  # noqa: E402

import concourse.bass as bass  # noqa: E402
import concourse.mybir as mybir  # noqa: E402
import concourse.tile as tile  # noqa: E402
from concourse import bacc  # noqa: E402
from concourse.bass import ts  # noqa: E402
from concourse.bass_utils import run_bass_kernel_spmd  # noqa: E402

S = 2048  # sequence length (S * X)
D = 1024  # model dim
H = 16  # total heads
HL = 4  # heads per core
DH = 64  # head dim
DQ = HL * DH  # per-core projection width = 256
NK = D // 128  # K tiles for projections = 8
NST = S // 128  # s_k tiles = 16
NCH = S // 512  # s_q chunks = 4

BF16 = mybir.dt.bfloat16
F32 = mybir.dt.float32

TRACE = False
LAST_RESULTS = None

_BUILT = None


def _emit(ctx, tc, io):
    nc = tc.nc
    xq, xk, xv = io["xqT"], io["xkT"], io["xvT"]
    wq, wk, wv, wo = io["wq"], io["wk"], io["wv"], io["wo"]
    bq, bk, bv = io["bq"], io["bk"], io["bv"]
    out = io["out"]

    consts = ctx.enter_context(tc.tile_pool(name="consts", bufs=1))
    xin = ctx.enter_context(tc.tile_pool(name="xin", bufs=1))
    qk = ctx.enter_context(tc.tile_pool(name="qk", bufs=1))
    ptiles = ctx.enter_context(tc.tile_pool(name="ptiles", bufs=5))
    norm = ctx.enter_context(tc.tile_pool(name="norm", bufs=3))
    osb_pool = ctx.enter_context(tc.tile_pool(name="osb", bufs=3))
    psum_mm = ctx.enter_context(tc.tile_pool(name="psum_mm", bufs=4, space="PSUM"))
    psum_lg = ctx.enter_context(tc.tile_pool(name="psum_lg", bufs=2, space="PSUM"))

    wq_t = [consts.tile([128, DQ], BF16, tag=f"wq{k}", name=f"wq{k}") for k in range(NK)]
    wk_t = [consts.tile([128, DQ], BF16, tag=f"wk{k}", name=f"wk{k}") for k in range(NK)]
    wv_t = [consts.tile([128, DQ], BF16, tag=f"wv{k}", name=f"wv{k}") for k in range(NK)]
    wo_t = [consts.tile([128, D], BF16, tag=f"wo{k}", name=f"wo{k}") for k in range(2)]
    xq_t = [xin.tile([128, S], BF16, tag=f"xq{k}", name=f"xq{k}") for k in range(NK)]
    xk_t = [xin.tile([128, S], BF16, tag=f"xk{k}", name=f"xk{k}") for k in range(NK)]
    xv_t = [xin.tile([128, S], BF16, tag=f"xv{k}", name=f"xv{k}") for k in range(NK)]
    # bq/bk as [128, 2] per-partition scalars (col j = dq 128j..128j+127)
    bq_sb = consts.tile([128, 2], F32, tag="bq", name="bq_sb")
    bk_sb = consts.tile([128, 2], F32, tag="bk", name="bk_sb")
    bv_sb = consts.tile([128, DQ], F32, tag="bv", name="bv_sb")

    # Tiny bias DMAs on the gpsimd (SWDGE) queue; all bulk traffic on the
    # sync (HWDGE) queue in strict dependency order: the kernel's front is
    # gated on (wk, xk) then (wq, xq); v-projection inputs come after.
    nc.gpsimd.dma_start(
        out=bk_sb[:], in_=bass.AP(tensor=bk.tensor, offset=bk.offset, ap=[[1, 128], [128, 2]])
    )
    nc.gpsimd.dma_start(
        out=bq_sb[:], in_=bass.AP(tensor=bq.tensor, offset=bq.offset, ap=[[1, 128], [128, 2]])
    )
    nc.gpsimd.dma_start(
        out=bv_sb[:], in_=bass.AP(tensor=bv.tensor, offset=bv.offset, ap=[[0, 128], [1, DQ]])
    )
    # x tensors stream in 512-column chunks, ordered by when the attention
    # pipeline consumes them: chunk-0 of everything first (attention starts
    # after ~1/4 of the input), then k/v chunks (consumed at 4 steps/chunk),
    # then the remaining q chunks (consumed at 32 steps/chunk).
    def dma_chunk(t, srcap, c):
        for k in range(NK):
            nc.sync.dma_start(t[k][:, ts(c, 512)], srcap[ts(k, 128), ts(c, 512)])

    for k in range(NK):
        nc.sync.dma_start(wk_t[k][:], wk[ts(k, 128), :])
    dma_chunk(xk_t, xk, 0)
    for k in range(NK):
        nc.sync.dma_start(wv_t[k][:], wv[ts(k, 128), :])
    dma_chunk(xv_t, xv, 0)
    for k in range(NK):
        nc.sync.dma_start(wq_t[k][:], wq[ts(k, 128), :])
    dma_chunk(xq_t, xq, 0)
    for c in range(1, NCH):
        dma_chunk(xk_t, xk, c)
        dma_chunk(xv_t, xv, c)
    for c in range(1, NCH):
        dma_chunk(xq_t, xq, c)
    for k in range(2):
        nc.sync.dma_start(wo_t[k][:], wo[ts(k, 128), :])

    # ---- projections: qT, kT = [256, 2048] as 2 tiles of [128, 2048] ----
    qT = [qk.tile([128, S], BF16, tag=f"qT{m}", name=f"qT{m}") for m in range(2)]
    kT = [qk.tile([128, S], BF16, tag=f"kT{m}", name=f"kT{m}") for m in range(2)]

    def qk_group(w_t, x_t, dst, b_sb, m, c, on_act):
        """One PSUM accumulation group of a q/k projection (+bias, ->bf16)."""
        ps = psum_mm.tile([128, 512], F32, tag="mm", name="mm")
        for k in range(NK):
            nc.tensor.matmul(
                ps[:],
                w_t[k][:, ts(m, 128)],
                x_t[k][:, ts(c, 512)],
                start=(k == 0),
                stop=(k == NK - 1),
            )
        if on_act:  # prefix phase: ScalarE is idle there
            nc.scalar.add(dst[m][:, ts(c, 512)], ps[:], b_sb[:, m : m + 1])
        else:  # injected into attention: keep ScalarE free for exp
            nc.vector.tensor_scalar_add(dst[m][:, ts(c, 512)], ps[:], b_sb[:, m : m + 1])

    # vaug[st] = [vA|1|vB|1|vC|1|vD|1]  [128, 260]
    vaug = [qk.tile([128, HL * (DH + 1)], BF16, tag=f"vaug{st}", name=f"vaug{st}") for st in range(NST)]

    def v_group(st):
        ps = psum_mm.tile([128, DQ], F32, tag="mm", name="mm")
        for k in range(NK):
            nc.tensor.matmul(
                ps[:],
                xv_t[k][:, ts(st, 128)],
                wv_t[k][:],
                start=(k == 0),
                stop=(k == NK - 1),
            )
        for h in range(HL):
            nc.vector.tensor_add(
                vaug[st][:, h * 65 : h * 65 + 64],
                ps[:, ts(h, DH)],
                bv_sb[:, ts(h, DH)],
            )
            nc.vector.memset(vaug[st][:, h * 65 + 64 : h * 65 + 65], 1.0)

    octT = [qk.tile([128, S], BF16, tag=f"octT{m}", name=f"octT{m}") for m in range(2)]

    def wo_group(c, smt, ncho, on_act=False):
        row = c * 512 + smt * 128
        ps = psum_mm.tile([128, 512], F32, tag="mm", name="mm")
        for k in range(2):
            nc.tensor.matmul(
                ps[:],
                octT[k][:, row : row + 128],
                wo_t[k][:, ts(ncho, 512)],
                start=(k == 0),
                stop=(k == 1),
            )
        osb = osb_pool.tile([128, 512], F32, tag="osb", name="osb")
        if on_act:  # tail drain: ScalarE is idle after the last exp
            nc.scalar.copy(osb[:], ps[:])
        else:
            nc.vector.tensor_copy(osb[:], ps[:])
        nc.sync.dma_start(out[row : row + 128, ts(ncho, 512)], osb[:])

    # ---- prefix: everything attention chunk 0 needs: all of kT and vaug,
    # qT chunk 0.  qT chunks 1-3 and all Wo groups are deferred into
    # pe_queue and drained inside the attention pipeline, hiding under
    # ScalarE exp time.
    # minimal prefix: only what attention step (c0, pr0, st0..3) needs.
    qk_group(wk_t, xk_t, kT, bk_sb, 0, 0, on_act=True)
    for st in range(4):
        v_group(st)
    qk_group(wq_t, xq_t, qT, bq_sb, 0, 0, on_act=True)

    # ---- attention: one flat software pipeline over all (c, pr, st)
    # steps.  The logits+exp front stream runs LAG steps ahead of the
    # accumulation stream, including across pair/chunk boundaries, so the
    # ScalarE exp stream never waits for a pipeline refill.
    steps = [(c, pr, st) for c in range(NCH) for pr in range(2) for st in range(NST)]
    LAG = 3

    def kg(m, c):
        return lambda: qk_group(wk_t, xk_t, kT, bk_sb, m, c, on_act=False)

    def qg(m, c):
        return lambda: qk_group(wq_t, xq_t, qT, bq_sb, m, c, on_act=False)

    def vg(st):
        return lambda: v_group(st)

    # deadline-ordered: kT chunks are consumed at 4 attention steps/chunk,
    # vaug at (step+LAG), the m=1 projections by step 16 (pair 1), q chunks
    # at 32 steps/chunk.
    pe_queue = [
        kg(0, 1), vg(4), vg(5), kg(0, 2), vg(6), vg(7), vg(8), kg(0, 3),
        vg(9), vg(10), vg(11), vg(12), kg(1, 0), qg(1, 0), vg(13), vg(14),
        vg(15), kg(1, 1), kg(1, 2), kg(1, 3),
    ]
    acc_map = {}
    p_map = {}

    def emit_front(c, pr, st):
        lg = psum_lg.tile([128, 1024], F32, tag="lg", name="lg")
        for hh in range(2):
            nc.tensor.matmul(
                lg[:, ts(hh, 512)],
                kT[pr][ts(hh, 64), ts(st, 128)],
                qT[pr][ts(hh, 64), ts(c, 512)],
                start=True,
                stop=True,
            )
        p = ptiles.tile([128, 1024], BF16, tag="p", name="p")
        nc.scalar.activation(p[:], lg[:], mybir.ActivationFunctionType.Exp, scale=0.125)
        p_map[(c, pr, st)] = p

    def emit_acc(c, pr, st):
        if st == 0:
            acc_map[(c, pr)] = [
                psum_mm.tile([65, 512], F32, tag="mm", name="mm") for _ in range(2)
            ]
        acc = acc_map[(c, pr)]
        pp = p_map.pop((c, pr, st))
        for hh in range(2):
            h = 2 * pr + hh
            nc.tensor.matmul(
                acc[hh][:],
                vaug[st][:, h * 65 : h * 65 + 65],
                pp[:, ts(hh, 512)],
                start=(st == 0),
                stop=(st == NST - 1),
            )
        if st == NST - 1:
            # normalize: octT[pr][64*hh, chunk c] = acc[0:64] / acc[64].
            # Fast copies release the PSUM accumulators; broadcast +
            # approx-reciprocal + mul run off the critical path.
            for hh in range(2):
                un = norm.tile([64, 512], BF16, tag="un", name="un")
                nc.vector.tensor_copy(un[:], acc[hh][0:64, :])
                den = norm.tile([1, 512], F32, tag="den", name="den")
                nc.vector.tensor_copy(den[:], acc[hh][64:65, :])
                bc = norm.tile([64, 512], F32, tag="bcast", name="bcast")
                nc.gpsimd.partition_broadcast(bc[:], den[:])
                rbc = norm.tile([64, 512], F32, tag="rbc", name="rbc")
                nc.vector.reciprocal_approx_fast(rbc[:], bc[:])
                nc.vector.tensor_mul(octT[pr][ts(hh, 64), ts(c, 512)], un[:], rbc[:])
            del acc_map[(c, pr)]
            if pr == 1:
                kwargs = {"on_act": True} if c == NCH - 1 else {}
                pe_queue.extend(
                    (lambda cc=c, smt=smt, ncho=ncho, kw=kwargs: wo_group(cc, smt, ncho, **kw))
                    for smt in range(4)
                    for ncho in range(2)
                )

    for i, s in enumerate(steps):
        c, pr, st = s
        if pr == 0 and st == 0 and c + 1 < NCH:
            pe_queue.extend(qg(m, c + 1) for m in range(2))
        emit_front(c, pr, st)
        if i >= LAG:
            emit_acc(*steps[i - LAG])
        if (c == 0 or st % 2 == 1) and pe_queue:
            pe_queue.pop(0)()

    for i in range(len(steps) - LAG, len(steps)):
        emit_acc(*steps[i])

    for g in pe_queue:
        g()


def _build():
    global _BUILT
    if _BUILT is not None:
        return _BUILT
    nc = bacc.Bacc(
        "TRN2",
        target_bir_lowering=False,
        debug=False,
        enable_asserts=False,
        num_devices=8,
    )
    io = {}
    io["xqT"] = nc.dram_tensor("xqT", [D, S], BF16, kind="ExternalInput").ap()
    io["xkT"] = nc.dram_tensor("xkT", [D, S], BF16, kind="ExternalInput").ap()
    io["xvT"] = nc.dram_tensor("xvT", [D, S], BF16, kind="ExternalInput").ap()
    io["wq"] = nc.dram_tensor("wq", [D, DQ], BF16, kind="ExternalInput").ap()
    io["wk"] = nc.dram_tensor("wk", [D, DQ], BF16, kind="ExternalInput").ap()
    io["wv"] = nc.dram_tensor("wv", [D, DQ], BF16, kind="ExternalInput").ap()
    io["wo"] = nc.dram_tensor("wo", [DQ, D], BF16, kind="ExternalInput").ap()
    io["bq"] = nc.dram_tensor("bq", [DQ], F32, kind="ExternalInput").ap()
    io["bk"] = nc.dram_tensor("bk", [DQ], F32, kind="ExternalInput").ap()
    io["bv"] = nc.dram_tensor("bv", [DQ], F32, kind="ExternalInput").ap()
    io["out"] = nc.dram_tensor("out", [S, D], F32, kind="ExternalOutput").ap()
    from contextlib import ExitStack

    with tile.TileContext(nc) as tc, ExitStack() as ctx:
        _emit(ctx, tc, io)
    nc.compile()
    _BUILT = nc
    return nc


def kernel(**inputs):
    global LAST_RESULTS
    bf16 = ml_dtypes.bfloat16
    query = np.asarray(inputs["query"], np.float32).reshape(2, S, D)
    key = np.asarray(inputs["key"], np.float32).reshape(2, S, D)
    value = np.asarray(inputs["value"], np.float32).reshape(2, S, D)
    Wq = np.asarray(inputs["Wq"], np.float32)
    Wk = np.asarray(inputs["Wk"], np.float32)
    Wv = np.asarray(inputs["Wv"], np.float32)
    Wo = np.asarray(inputs["Wo"], np.float32)
    bq = np.asarray(inputs["bq"], np.float32)
    bk = np.asarray(inputs["bk"], np.float32)
    bv = np.asarray(inputs["bv"], np.float32)
    bo = np.asarray(inputs["bo"], np.float32)

    xT = {}
    for b in range(2):
        xT[("q", b)] = np.ascontiguousarray(query[b].T).astype(bf16)
        xT[("k", b)] = np.ascontiguousarray(key[b].T).astype(bf16)
        xT[("v", b)] = np.ascontiguousarray(value[b].T).astype(bf16)

    in_maps = []
    for c in range(8):
        b, g = c // 4, c % 4
        sl = slice(g * DQ, (g + 1) * DQ)
        in_maps.append(
            {
                "xqT": xT[("q", b)],
                "xkT": xT[("k", b)],
                "xvT": xT[("v", b)],
                "wq": np.ascontiguousarray(Wq[:, sl]).astype(bf16),
                "wk": np.ascontiguousarray(Wk[:, sl]).astype(bf16),
                "wv": np.ascontiguousarray(Wv[:, sl]).astype(bf16),
                "wo": np.ascontiguousarray(Wo[sl, :]).astype(bf16),
                "bq": np.ascontiguousarray(bq[sl]),
                "bk": np.ascontiguousarray(bk[sl]),
                "bv": np.ascontiguousarray(bv[sl]),
            }
        )

    nc = _build()
    res = run_bass_kernel_spmd(
        nc, in_maps, core_ids=list(range(8)), trace=TRACE
    )
    LAST_RESULTS = res

    full = np.zeros((2, S, D), np.float32)
    for c in range(8):
        full[c // 4] += res.results[c]["out"]
    full += bo[None, None, :]
    return full.reshape(2, S, 1, D)


# revision 17
# speedup vs baseline: 1.0418x; 1.0418x over previous
"""MultiHeadAttention Trainium2 kernel (8 NeuronCores, SPMD).

Sharding: data-parallel over batch (B=2), tensor-parallel over heads
(16 heads -> 4 per core).  Core c handles batch b=c//4, head group
g=c%4 (heads 4g..4g+3).  Wq/Wk/Wv are split column-wise, Wo row-wise;
the per-core Wo partial outputs are summed on the host (replaces the
all-reduce).

Device dataflow per core (bf16 matmuls, f32 PSUM accumulation):
  qT = Wq_g^T x^T   [256, 2048]   (heads on partitions, dh=64 each)
  kT = Wk_g^T x^T   [256, 2048]
  v  = x Wv_g       [2048, 256] stored interleaved with a ones column
                    per head: vaug[st] = [vA|1|vB|1|vC|1|vD|1]
  per (s_q chunk of 512, head pair):
    logitsT[s_k, s_q] = kT^T qT / 8       (two heads packed in PE row
                                           groups, K=64 each)
    p = exp(logitsT)  on ScalarE, scale=1/8 fused, bf16 out
    accT[65, s_q] += vaug_h^T p           (row 64 = softmax denominator)
    outcatT[h] = accT[0:64] * bcast(1/accT[64])   (deferred softmax norm)
  partial = outcatT^T Wo_g  -> DRAM f32
"""

import sys

import numpy as np

sys.path.insert(0, "/opt/trn_rl_repo")

import ml_dtypes  # noqa: E402

import concourse.bass as bass  # noqa: E402
import concourse.mybir as mybir  # noqa: E402
import concourse.tile as tile  # noqa: E402
from concourse import bacc  # noqa: E402
from concourse.bass import ts  # noqa: E402
from concourse.bass_utils import run_bass_kernel_spmd  # noqa: E402

S = 2048  # sequence length (S * X)
D = 1024  # model dim
H = 16  # total heads
HL = 4  # heads per core
DH = 64  # head dim
DQ = HL * DH  # per-core projection width = 256
NK = D // 128  # K tiles for projections = 8
NST = S // 128  # s_k tiles = 16
NCH = S // 512  # s_q chunks = 4

BF16 = mybir.dt.bfloat16
F32 = mybir.dt.float32

TRACE = False
LAST_RESULTS = None

_BUILT = None


def _emit(ctx, tc, io):
    nc = tc.nc
    xq, xk, xv = io["xqT"], io["xkT"], io["xvT"]
    wq, wk, wv, wo = io["wq"], io["wk"], io["wv"], io["wo"]
    bq, bk, bv = io["bq"], io["bk"], io["bv"]
    out = io["out"]

    consts = ctx.enter_context(tc.tile_pool(name="consts", bufs=1))
    xin = ctx.enter_context(tc.tile_pool(name="xin", bufs=1))
    qk = ctx.enter_context(tc.tile_pool(name="qk", bufs=1))
    ptiles = ctx.enter_context(tc.tile_pool(name="ptiles", bufs=4))
    norm = ctx.enter_context(tc.tile_pool(name="norm", bufs=3))
    osb_pool = ctx.enter_context(tc.tile_pool(name="osb", bufs=3))
    psum_mm = ctx.enter_context(tc.tile_pool(name="psum_mm", bufs=4, space="PSUM"))
    psum_lg = ctx.enter_context(tc.tile_pool(name="psum_lg", bufs=2, space="PSUM"))

    wq_t = [consts.tile([128, DQ], BF16, tag=f"wq{k}", name=f"wq{k}") for k in range(NK)]
    wk_t = [consts.tile([128, DQ], BF16, tag=f"wk{k}", name=f"wk{k}") for k in range(NK)]
    wv_t = [consts.tile([128, DQ], BF16, tag=f"wv{k}", name=f"wv{k}") for k in range(NK)]
    wo_t = [consts.tile([128, D], BF16, tag=f"wo{k}", name=f"wo{k}") for k in range(2)]
    xq_t = [xin.tile([128, S], BF16, tag=f"xq{k}", name=f"xq{k}") for k in range(NK)]
    xk_t = [xin.tile([128, S], BF16, tag=f"xk{k}", name=f"xk{k}") for k in range(NK)]
    xv_t = [xin.tile([128, S], BF16, tag=f"xv{k}", name=f"xv{k}") for k in range(NK)]
    # bq/bk as [128, 2] per-partition scalars (col j = dq 128j..128j+127)
    bq_sb = consts.tile([128, 2], F32, tag="bq", name="bq_sb")
    bk_sb = consts.tile([128, 2], F32, tag="bk", name="bk_sb")
    bv_sb = consts.tile([128, DQ], F32, tag="bv", name="bv_sb")

    # Tiny bias DMAs on the gpsimd (SWDGE) queue; all bulk traffic on the
    # sync (HWDGE) queue in dependency order (k-projection inputs first,
    # then v, then q, Wo last).
    nc.gpsimd.dma_start(
        out=bk_sb[:], in_=bass.AP(tensor=bk.tensor, offset=bk.offset, ap=[[1, 128], [128, 2]])
    )
    nc.gpsimd.dma_start(
        out=bq_sb[:], in_=bass.AP(tensor=bq.tensor, offset=bq.offset, ap=[[1, 128], [128, 2]])
    )
    nc.gpsimd.dma_start(
        out=bv_sb[:], in_=bass.AP(tensor=bv.tensor, offset=bv.offset, ap=[[0, 128], [1, DQ]])
    )
    for k in range(NK):
        nc.sync.dma_start(wk_t[k][:], wk[ts(k, 128), :])
    for k in range(NK):
        nc.sync.dma_start(xk_t[k][:], xk[ts(k, 128), :])
    for k in range(NK):
        nc.sync.dma_start(wv_t[k][:], wv[ts(k, 128), :])
    for k in range(NK):
        nc.sync.dma_start(xv_t[k][:], xv[ts(k, 128), :])
    for k in range(NK):
        nc.sync.dma_start(wq_t[k][:], wq[ts(k, 128), :])
    for k in range(NK):
        nc.sync.dma_start(xq_t[k][:], xq[ts(k, 128), :])
    for k in range(2):
        nc.sync.dma_start(wo_t[k][:], wo[ts(k, 128), :])

    # ---- projections: qT, kT = [256, 2048] as 2 tiles of [128, 2048] ----
    qT = [qk.tile([128, S], BF16, tag=f"qT{m}", name=f"qT{m}") for m in range(2)]
    kT = [qk.tile([128, S], BF16, tag=f"kT{m}", name=f"kT{m}") for m in range(2)]

    def qk_group(w_t, x_t, dst, b_sb, m, c, on_act):
        """One PSUM accumulation group of a q/k projection (+bias, ->bf16)."""
        ps = psum_mm.tile([128, 512], F32, tag="mm", name="mm")
        for k in range(NK):
            nc.tensor.matmul(
                ps[:],
                w_t[k][:, ts(m, 128)],
                x_t[k][:, ts(c, 512)],
                start=(k == 0),
                stop=(k == NK - 1),
            )
        if on_act:  # prefix phase: ScalarE is idle there
            nc.scalar.add(dst[m][:, ts(c, 512)], ps[:], b_sb[:, m : m + 1])
        else:  # injected into attention: keep ScalarE free for exp
            nc.vector.tensor_scalar_add(dst[m][:, ts(c, 512)], ps[:], b_sb[:, m : m + 1])

    # vaug[st] = [vA|1|vB|1|vC|1|vD|1]  [128, 260]
    vaug = [qk.tile([128, HL * (DH + 1)], BF16, tag=f"vaug{st}", name=f"vaug{st}") for st in range(NST)]

    def v_block(blk):
        # 4 st tiles per block, k outermost: each k-pass runs as soon as
        # xv tile k lands, so v-projection paces with its DMA instead of
        # serializing after it.
        sts = range(blk * 4, blk * 4 + 4)
        pss = {st: psum_mm.tile([128, DQ], F32, tag="mm", name="mm") for st in sts}
        for k in range(NK):
            for st in sts:
                nc.tensor.matmul(
                    pss[st][:],
                    xv_t[k][:, ts(st, 128)],
                    wv_t[k][:],
                    start=(k == 0),
                    stop=(k == NK - 1),
                )
        for st in sts:
            for h in range(HL):
                nc.vector.tensor_add(
                    vaug[st][:, h * 65 : h * 65 + 64],
                    pss[st][:, ts(h, DH)],
                    bv_sb[:, ts(h, DH)],
                )
                nc.vector.memset(vaug[st][:, h * 65 + 64 : h * 65 + 65], 1.0)

    octT = [qk.tile([128, S], BF16, tag=f"octT{m}", name=f"octT{m}") for m in range(2)]

    def wo_group(c, smt, ncho, on_act=False):
        row = c * 512 + smt * 128
        ps = psum_mm.tile([128, 512], F32, tag="mm", name="mm")
        for k in range(2):
            nc.tensor.matmul(
                ps[:],
                octT[k][:, row : row + 128],
                wo_t[k][:, ts(ncho, 512)],
                start=(k == 0),
                stop=(k == 1),
            )
        osb = osb_pool.tile([128, 512], F32, tag="osb", name="osb")
        if on_act:  # tail drain: ScalarE is idle after the last exp
            nc.scalar.copy(osb[:], ps[:])
        else:
            nc.vector.tensor_copy(osb[:], ps[:])
        nc.sync.dma_start(out[row : row + 128, ts(ncho, 512)], osb[:])

    # ---- prefix: everything attention chunk 0 needs: all of kT and vaug,
    # qT chunk 0.  qT chunks 1-3 and all Wo groups are deferred into
    # pe_queue and drained inside the attention pipeline, hiding under
    # ScalarE exp time.
    for m in range(2):
        for c in range(NCH):
            qk_group(wk_t, xk_t, kT, bk_sb, m, c, on_act=True)
    for blk in range(4):
        v_block(blk)
    for m in range(2):
        qk_group(wq_t, xq_t, qT, bq_sb, m, 0, on_act=True)

    # ---- attention: one flat software pipeline over all (c, pr, st)
    # steps.  The logits+exp front stream runs LAG steps ahead of the
    # accumulation stream, including across pair/chunk boundaries, so the
    # ScalarE exp stream never waits for a pipeline refill.
    steps = [(c, pr, st) for c in range(NCH) for pr in range(2) for st in range(NST)]
    LAG = 2

    def qg(m, c):
        return lambda: qk_group(wq_t, xq_t, qT, bq_sb, m, c, on_act=False)

    pe_queue = []
    acc_map = {}
    p_map = {}

    def emit_front(c, pr, st):
        lg = psum_lg.tile([128, 1024], F32, tag="lg", name="lg")
        for hh in range(2):
            nc.tensor.matmul(
                lg[:, ts(hh, 512)],
                kT[pr][ts(hh, 64), ts(st, 128)],
                qT[pr][ts(hh, 64), ts(c, 512)],
                start=True,
                stop=True,
            )
        p = ptiles.tile([128, 1024], BF16, tag="p", name="p")
        nc.scalar.activation(p[:], lg[:], mybir.ActivationFunctionType.Exp, scale=0.125)
        p_map[(c, pr, st)] = p

    def emit_acc(c, pr, st):
        if st == 0:
            acc_map[(c, pr)] = [
                psum_mm.tile([65, 512], F32, tag="mm", name="mm") for _ in range(2)
            ]
        acc = acc_map[(c, pr)]
        pp = p_map.pop((c, pr, st))
        for hh in range(2):
            h = 2 * pr + hh
            nc.tensor.matmul(
                acc[hh][:],
                vaug[st][:, h * 65 : h * 65 + 65],
                pp[:, ts(hh, 512)],
                start=(st == 0),
                stop=(st == NST - 1),
            )
        if st == NST - 1:
            # normalize: octT[pr][64*hh, chunk c] = acc[0:64] / acc[64].
            # Fast copies release the PSUM accumulators; broadcast +
            # approx-reciprocal + mul run off the critical path.
            for hh in range(2):
                un = norm.tile([64, 512], BF16, tag="un", name="un")
                nc.vector.tensor_copy(un[:], acc[hh][0:64, :])
                den = norm.tile([1, 512], F32, tag="den", name="den")
                nc.vector.tensor_copy(den[:], acc[hh][64:65, :])
                bc = norm.tile([64, 512], F32, tag="bcast", name="bcast")
                nc.gpsimd.partition_broadcast(bc[:], den[:])
                rbc = norm.tile([64, 512], F32, tag="rbc", name="rbc")
                nc.vector.reciprocal_approx_fast(rbc[:], bc[:])
                nc.vector.tensor_mul(octT[pr][ts(hh, 64), ts(c, 512)], un[:], rbc[:])
            del acc_map[(c, pr)]
            if pr == 1:
                kwargs = {"on_act": True} if c == NCH - 1 else {}
                pe_queue.extend(
                    (lambda cc=c, smt=smt, ncho=ncho, kw=kwargs: wo_group(cc, smt, ncho, **kw))
                    for smt in range(4)
                    for ncho in range(2)
                )

    for i, s in enumerate(steps):
        c, pr, st = s
        if pr == 0 and st == 0 and c + 1 < NCH:
            pe_queue.extend(qg(m, c + 1) for m in range(2))
        emit_front(c, pr, st)
        if i >= LAG:
            emit_acc(*steps[i - LAG])
        if st % 2 == 1 and pe_queue:
            pe_queue.pop(0)()

    for i in range(len(steps) - LAG, len(steps)):
        emit_acc(*steps[i])

    for g in pe_queue:
        g()


def _build():
    global _BUILT
    if _BUILT is not None:
        return _BUILT
    nc = bacc.Bacc(
        "TRN2",
        target_bir_lowering=False,
        debug=False,
        enable_asserts=False,
        num_devices=8,
    )
    io = {}
    io["xqT"] = nc.dram_tensor("xqT", [D, S], BF16, kind="ExternalInput").ap()
    io["xkT"] = nc.dram_tensor("xkT", [D, S], BF16, kind="ExternalInput").ap()
    io["xvT"] = nc.dram_tensor("xvT", [D, S], BF16, kind="ExternalInput").ap()
    io["wq"] = nc.dram_tensor("wq", [D, DQ], BF16, kind="ExternalInput").ap()
    io["wk"] = nc.dram_tensor("wk", [D, DQ], BF16, kind="ExternalInput").ap()
    io["wv"] = nc.dram_tensor("wv", [D, DQ], BF16, kind="ExternalInput").ap()
    io["wo"] = nc.dram_tensor("wo", [DQ, D], BF16, kind="ExternalInput").ap()
    io["bq"] = nc.dram_tensor("bq", [DQ], F32, kind="ExternalInput").ap()
    io["bk"] = nc.dram_tensor("bk", [DQ], F32, kind="ExternalInput").ap()
    io["bv"] = nc.dram_tensor("bv", [DQ], F32, kind="ExternalInput").ap()
    io["out"] = nc.dram_tensor("out", [S, D], F32, kind="ExternalOutput").ap()
    from contextlib import ExitStack

    with tile.TileContext(nc) as tc, ExitStack() as ctx:
        _emit(ctx, tc, io)
    nc.compile()
    _BUILT = nc
    return nc


def kernel(**inputs):
    global LAST_RESULTS
    bf16 = ml_dtypes.bfloat16
    query = np.asarray(inputs["query"], np.float32).reshape(2, S, D)
    key = np.asarray(inputs["key"], np.float32).reshape(2, S, D)
    value = np.asarray(inputs["value"], np.float32).reshape(2, S, D)
    Wq = np.asarray(inputs["Wq"], np.float32)
    Wk = np.asarray(inputs["Wk"], np.float32)
    Wv = np.asarray(inputs["Wv"], np.float32)
    Wo = np.asarray(inputs["Wo"], np.float32)
    bq = np.asarray(inputs["bq"], np.float32)
    bk = np.asarray(inputs["bk"], np.float32)
    bv = np.asarray(inputs["bv"], np.float32)
    bo = np.asarray(inputs["bo"], np.float32)

    xT = {}
    for b in range(2):
        xT[("q", b)] = np.ascontiguousarray(query[b].T).astype(bf16)
        xT[("k", b)] = np.ascontiguousarray(key[b].T).astype(bf16)
        xT[("v", b)] = np.ascontiguousarray(value[b].T).astype(bf16)

    in_maps = []
    for c in range(8):
        b, g = c // 4, c % 4
        sl = slice(g * DQ, (g + 1) * DQ)
        in_maps.append(
            {
                "xqT": xT[("q", b)],
                "xkT": xT[("k", b)],
                "xvT": xT[("v", b)],
                "wq": np.ascontiguousarray(Wq[:, sl]).astype(bf16),
                "wk": np.ascontiguousarray(Wk[:, sl]).astype(bf16),
                "wv": np.ascontiguousarray(Wv[:, sl]).astype(bf16),
                "wo": np.ascontiguousarray(Wo[sl, :]).astype(bf16),
                "bq": np.ascontiguousarray(bq[sl]),
                "bk": np.ascontiguousarray(bk[sl]),
                "bv": np.ascontiguousarray(bv[sl]),
            }
        )

    nc = _build()
    res = run_bass_kernel_spmd(
        nc, in_maps, core_ids=list(range(8)), trace=TRACE
    )
    LAST_RESULTS = res

    full = np.zeros((2, S, D), np.float32)
    for c in range(8):
        full[c // 4] += res.results[c]["out"]
    full += bo[None, None, :]
    return full.reshape(2, S, 1, D)


# revision 18
# speedup vs baseline: 1.0463x; 1.0043x over previous
"""MultiHeadAttention Trainium2 kernel (8 NeuronCores, SPMD).

Sharding: data-parallel over batch (B=2), tensor-parallel over heads
(16 heads -> 4 per core).  Core c handles batch b=c//4, head group
g=c%4 (heads 4g..4g+3).  Wq/Wk/Wv are split column-wise, Wo row-wise;
the per-core Wo partial outputs are summed on the host (replaces the
all-reduce).

Device dataflow per core (bf16 matmuls, f32 PSUM accumulation):
  qT = Wq_g^T x^T   [256, 2048]   (heads on partitions, dh=64 each)
  kT = Wk_g^T x^T   [256, 2048]
  v  = x Wv_g       [2048, 256] stored interleaved with a ones column
                    per head: vaug[st] = [vA|1|vB|1|vC|1|vD|1]
  per (s_q chunk of 512, head pair):
    logitsT[s_k, s_q] = kT^T qT / 8       (two heads packed in PE row
                                           groups, K=64 each)
    p = exp(logitsT)  on ScalarE, scale=1/8 fused, bf16 out
    accT[65, s_q] += vaug_h^T p           (row 64 = softmax denominator)
    outcatT[h] = accT[0:64] * bcast(1/accT[64])   (deferred softmax norm)
  partial = outcatT^T Wo_g  -> DRAM f32
"""

import sys

import numpy as np

sys.path.insert(0, "/opt/trn_rl_repo")

import ml_dtypes  # noqa: E402

import concourse.bass as bass  # noqa: E402
import concourse.mybir as mybir  # noqa: E402
import concourse.tile as tile  # noqa: E402
from concourse import bacc  # noqa: E402
from concourse.bass import ts  # noqa: E402
from concourse.bass_utils import run_bass_kernel_spmd  # noqa: E402

S = 2048  # sequence length (S * X)
D = 1024  # model dim
H = 16  # total heads
HL = 4  # heads per core
DH = 64  # head dim
DQ = HL * DH  # per-core projection width = 256
NK = D // 128  # K tiles for projections = 8
NST = S // 128  # s_k tiles = 16
NCH = S // 512  # s_q chunks = 4

BF16 = mybir.dt.bfloat16
F32 = mybir.dt.float32

TRACE = False
LAST_RESULTS = None

_BUILT = None


def _emit(ctx, tc, io):
    nc = tc.nc
    xq, xk, xv = io["xqT"], io["xkT"], io["xvT"]
    wq, wk, wv, wo = io["wq"], io["wk"], io["wv"], io["wo"]
    bq, bk, bv = io["bq"], io["bk"], io["bv"]
    out = io["out"]

    consts = ctx.enter_context(tc.tile_pool(name="consts", bufs=1))
    xin = ctx.enter_context(tc.tile_pool(name="xin", bufs=1))
    qk = ctx.enter_context(tc.tile_pool(name="qk", bufs=1))
    ptiles = ctx.enter_context(tc.tile_pool(name="ptiles", bufs=4))
    norm = ctx.enter_context(tc.tile_pool(name="norm", bufs=3))
    osb_pool = ctx.enter_context(tc.tile_pool(name="osb", bufs=3))
    psum_mm = ctx.enter_context(tc.tile_pool(name="psum_mm", bufs=4, space="PSUM"))
    psum_lg = ctx.enter_context(tc.tile_pool(name="psum_lg", bufs=2, space="PSUM"))

    wq_t = [consts.tile([128, DQ], BF16, tag=f"wq{k}", name=f"wq{k}") for k in range(NK)]
    wk_t = [consts.tile([128, DQ], BF16, tag=f"wk{k}", name=f"wk{k}") for k in range(NK)]
    wv_t = [consts.tile([128, DQ], BF16, tag=f"wv{k}", name=f"wv{k}") for k in range(NK)]
    wo_t = [consts.tile([128, D], BF16, tag=f"wo{k}", name=f"wo{k}") for k in range(2)]
    xq_t = [xin.tile([128, S], BF16, tag=f"xq{k}", name=f"xq{k}") for k in range(NK)]
    xk_t = [xin.tile([128, S], BF16, tag=f"xk{k}", name=f"xk{k}") for k in range(NK)]
    xv_t = [xin.tile([128, S], BF16, tag=f"xv{k}", name=f"xv{k}") for k in range(NK)]
    # bq/bk as [128, 2] per-partition scalars (col j = dq 128j..128j+127)
    bq_sb = consts.tile([128, 2], F32, tag="bq", name="bq_sb")
    bk_sb = consts.tile([128, 2], F32, tag="bk", name="bk_sb")
    bv_sb = consts.tile([128, DQ], F32, tag="bv", name="bv_sb")

    # PE warmup: ~10us of dummy back-to-back matmuls at t=0, while the PE
    # would otherwise sit idle waiting for input DMA.  The PE clock gate
    # (HAM) defaults to 4/8 throttle (1.2 GHz) and only releases after
    # ~3.4us of sustained activity; without this the whole DMA-paced
    # front (k/v/q projections) runs at half clock.
    wu_sb = consts.tile([128, 512], BF16, tag="wu", name="wu_sb")
    nc.vector.memset(wu_sb[:], 1.0)
    wu_ps = psum_lg.tile([128, 512], F32, tag="lg", name="lg")
    for _ in range(32):
        nc.tensor.matmul(wu_ps[:], wu_sb[:, 0:128], wu_sb[:], start=True, stop=True)

    # Tiny bias DMAs on the gpsimd (SWDGE) queue; all bulk traffic on the
    # sync (HWDGE) queue in dependency order (k-projection inputs first,
    # then v, then q, Wo last).
    nc.gpsimd.dma_start(
        out=bk_sb[:], in_=bass.AP(tensor=bk.tensor, offset=bk.offset, ap=[[1, 128], [128, 2]])
    )
    nc.gpsimd.dma_start(
        out=bq_sb[:], in_=bass.AP(tensor=bq.tensor, offset=bq.offset, ap=[[1, 128], [128, 2]])
    )
    nc.gpsimd.dma_start(
        out=bv_sb[:], in_=bass.AP(tensor=bv.tensor, offset=bv.offset, ap=[[0, 128], [1, DQ]])
    )
    for k in range(NK):
        nc.sync.dma_start(wk_t[k][:], wk[ts(k, 128), :])
    for k in range(NK):
        nc.sync.dma_start(xk_t[k][:], xk[ts(k, 128), :])
    for k in range(NK):
        nc.sync.dma_start(wv_t[k][:], wv[ts(k, 128), :])
    for k in range(NK):
        nc.sync.dma_start(xv_t[k][:], xv[ts(k, 128), :])
    for k in range(NK):
        nc.sync.dma_start(wq_t[k][:], wq[ts(k, 128), :])
    for k in range(NK):
        nc.sync.dma_start(xq_t[k][:], xq[ts(k, 128), :])
    for k in range(2):
        nc.sync.dma_start(wo_t[k][:], wo[ts(k, 128), :])

    # ---- projections: qT, kT = [256, 2048] as 2 tiles of [128, 2048] ----
    qT = [qk.tile([128, S], BF16, tag=f"qT{m}", name=f"qT{m}") for m in range(2)]
    kT = [qk.tile([128, S], BF16, tag=f"kT{m}", name=f"kT{m}") for m in range(2)]

    def qk_group(w_t, x_t, dst, b_sb, m, c, on_act):
        """One PSUM accumulation group of a q/k projection (+bias, ->bf16)."""
        ps = psum_mm.tile([128, 512], F32, tag="mm", name="mm")
        for k in range(NK):
            nc.tensor.matmul(
                ps[:],
                w_t[k][:, ts(m, 128)],
                x_t[k][:, ts(c, 512)],
                start=(k == 0),
                stop=(k == NK - 1),
            )
        if on_act:  # prefix phase: ScalarE is idle there
            nc.scalar.add(dst[m][:, ts(c, 512)], ps[:], b_sb[:, m : m + 1])
        else:  # injected into attention: keep ScalarE free for exp
            nc.vector.tensor_scalar_add(dst[m][:, ts(c, 512)], ps[:], b_sb[:, m : m + 1])

    # vaug[st] = [vA|1|vB|1|vC|1|vD|1]  [128, 260]
    vaug = [qk.tile([128, HL * (DH + 1)], BF16, tag=f"vaug{st}", name=f"vaug{st}") for st in range(NST)]

    def v_block(blk):
        # 4 st tiles per block, k outermost: each k-pass runs as soon as
        # xv tile k lands, so v-projection paces with its DMA instead of
        # serializing after it.
        sts = range(blk * 4, blk * 4 + 4)
        pss = {st: psum_mm.tile([128, DQ], F32, tag="mm", name="mm") for st in sts}
        for k in range(NK):
            for st in sts:
                nc.tensor.matmul(
                    pss[st][:],
                    xv_t[k][:, ts(st, 128)],
                    wv_t[k][:],
                    start=(k == 0),
                    stop=(k == NK - 1),
                )
        for st in sts:
            for h in range(HL):
                nc.vector.tensor_add(
                    vaug[st][:, h * 65 : h * 65 + 64],
                    pss[st][:, ts(h, DH)],
                    bv_sb[:, ts(h, DH)],
                )
                nc.vector.memset(vaug[st][:, h * 65 + 64 : h * 65 + 65], 1.0)

    octT = [qk.tile([128, S], BF16, tag=f"octT{m}", name=f"octT{m}") for m in range(2)]

    def wo_group(c, smt, ncho, on_act=False):
        row = c * 512 + smt * 128
        ps = psum_mm.tile([128, 512], F32, tag="mm", name="mm")
        for k in range(2):
            nc.tensor.matmul(
                ps[:],
                octT[k][:, row : row + 128],
                wo_t[k][:, ts(ncho, 512)],
                start=(k == 0),
                stop=(k == 1),
            )
        osb = osb_pool.tile([128, 512], F32, tag="osb", name="osb")
        if on_act:  # tail drain: ScalarE is idle after the last exp
            nc.scalar.copy(osb[:], ps[:])
        else:
            nc.vector.tensor_copy(osb[:], ps[:])
        nc.sync.dma_start(out[row : row + 128, ts(ncho, 512)], osb[:])

    # ---- prefix: everything attention chunk 0 needs: all of kT and vaug,
    # qT chunk 0.  qT chunks 1-3 and all Wo groups are deferred into
    # pe_queue and drained inside the attention pipeline, hiding under
    # ScalarE exp time.
    for m in range(2):
        for c in range(NCH):
            qk_group(wk_t, xk_t, kT, bk_sb, m, c, on_act=True)
    for blk in range(4):
        v_block(blk)
    for m in range(2):
        qk_group(wq_t, xq_t, qT, bq_sb, m, 0, on_act=True)

    # ---- attention: one flat software pipeline over all (c, pr, st)
    # steps.  The logits+exp front stream runs LAG steps ahead of the
    # accumulation stream, including across pair/chunk boundaries, so the
    # ScalarE exp stream never waits for a pipeline refill.
    steps = [(c, pr, st) for c in range(NCH) for pr in range(2) for st in range(NST)]
    LAG = 2

    def qg(m, c):
        return lambda: qk_group(wq_t, xq_t, qT, bq_sb, m, c, on_act=False)

    pe_queue = []
    acc_map = {}
    p_map = {}

    def emit_front(c, pr, st):
        lg = psum_lg.tile([128, 1024], F32, tag="lg", name="lg")
        for hh in range(2):
            nc.tensor.matmul(
                lg[:, ts(hh, 512)],
                kT[pr][ts(hh, 64), ts(st, 128)],
                qT[pr][ts(hh, 64), ts(c, 512)],
                start=True,
                stop=True,
            )
        p = ptiles.tile([128, 1024], BF16, tag="p", name="p")
        nc.scalar.activation(p[:], lg[:], mybir.ActivationFunctionType.Exp, scale=0.125)
        p_map[(c, pr, st)] = p

    def emit_acc(c, pr, st):
        if st == 0:
            acc_map[(c, pr)] = [
                psum_mm.tile([65, 512], F32, tag="mm", name="mm") for _ in range(2)
            ]
        acc = acc_map[(c, pr)]
        pp = p_map.pop((c, pr, st))
        for hh in range(2):
            h = 2 * pr + hh
            nc.tensor.matmul(
                acc[hh][:],
                vaug[st][:, h * 65 : h * 65 + 65],
                pp[:, ts(hh, 512)],
                start=(st == 0),
                stop=(st == NST - 1),
            )
        if st == NST - 1:
            # normalize: octT[pr][64*hh, chunk c] = acc[0:64] / acc[64].
            # Fast copies release the PSUM accumulators; broadcast +
            # approx-reciprocal + mul run off the critical path.
            for hh in range(2):
                un = norm.tile([64, 512], BF16, tag="un", name="un")
                nc.vector.tensor_copy(un[:], acc[hh][0:64, :])
                den = norm.tile([1, 512], F32, tag="den", name="den")
                nc.vector.tensor_copy(den[:], acc[hh][64:65, :])
                bc = norm.tile([64, 512], F32, tag="bcast", name="bcast")
                nc.gpsimd.partition_broadcast(bc[:], den[:])
                rbc = norm.tile([64, 512], F32, tag="rbc", name="rbc")
                nc.vector.reciprocal_approx_fast(rbc[:], bc[:])
                nc.vector.tensor_mul(octT[pr][ts(hh, 64), ts(c, 512)], un[:], rbc[:])
            del acc_map[(c, pr)]
            if pr == 1:
                kwargs = {"on_act": True} if c == NCH - 1 else {}
                pe_queue.extend(
                    (lambda cc=c, smt=smt, ncho=ncho, kw=kwargs: wo_group(cc, smt, ncho, **kw))
                    for smt in range(4)
                    for ncho in range(2)
                )

    for i, s in enumerate(steps):
        c, pr, st = s
        if pr == 0 and st == 0 and c + 1 < NCH:
            pe_queue.extend(qg(m, c + 1) for m in range(2))
        emit_front(c, pr, st)
        if i >= LAG:
            emit_acc(*steps[i - LAG])
        if st % 2 == 1 and pe_queue:
            pe_queue.pop(0)()

    for i in range(len(steps) - LAG, len(steps)):
        emit_acc(*steps[i])

    for g in pe_queue:
        g()


def _build():
    global _BUILT
    if _BUILT is not None:
        return _BUILT
    nc = bacc.Bacc(
        "TRN2",
        target_bir_lowering=False,
        debug=False,
        enable_asserts=False,
        num_devices=8,
    )
    io = {}
    io["xqT"] = nc.dram_tensor("xqT", [D, S], BF16, kind="ExternalInput").ap()
    io["xkT"] = nc.dram_tensor("xkT", [D, S], BF16, kind="ExternalInput").ap()
    io["xvT"] = nc.dram_tensor("xvT", [D, S], BF16, kind="ExternalInput").ap()
    io["wq"] = nc.dram_tensor("wq", [D, DQ], BF16, kind="ExternalInput").ap()
    io["wk"] = nc.dram_tensor("wk", [D, DQ], BF16, kind="ExternalInput").ap()
    io["wv"] = nc.dram_tensor("wv", [D, DQ], BF16, kind="ExternalInput").ap()
    io["wo"] = nc.dram_tensor("wo", [DQ, D], BF16, kind="ExternalInput").ap()
    io["bq"] = nc.dram_tensor("bq", [DQ], F32, kind="ExternalInput").ap()
    io["bk"] = nc.dram_tensor("bk", [DQ], F32, kind="ExternalInput").ap()
    io["bv"] = nc.dram_tensor("bv", [DQ], F32, kind="ExternalInput").ap()
    io["out"] = nc.dram_tensor("out", [S, D], F32, kind="ExternalOutput").ap()
    from contextlib import ExitStack

    with tile.TileContext(nc) as tc, ExitStack() as ctx:
        _emit(ctx, tc, io)
    nc.compile()
    _BUILT = nc
    return nc


def kernel(**inputs):
    global LAST_RESULTS
    bf16 = ml_dtypes.bfloat16
    query = np.asarray(inputs["query"], np.float32).reshape(2, S, D)
    key = np.asarray(inputs["key"], np.float32).reshape(2, S, D)
    value = np.asarray(inputs["value"], np.float32).reshape(2, S, D)
    Wq = np.asarray(inputs["Wq"], np.float32)
    Wk = np.asarray(inputs["Wk"], np.float32)
    Wv = np.asarray(inputs["Wv"], np.float32)
    Wo = np.asarray(inputs["Wo"], np.float32)
    bq = np.asarray(inputs["bq"], np.float32)
    bk = np.asarray(inputs["bk"], np.float32)
    bv = np.asarray(inputs["bv"], np.float32)
    bo = np.asarray(inputs["bo"], np.float32)

    xT = {}
    for b in range(2):
        xT[("q", b)] = np.ascontiguousarray(query[b].T).astype(bf16)
        xT[("k", b)] = np.ascontiguousarray(key[b].T).astype(bf16)
        xT[("v", b)] = np.ascontiguousarray(value[b].T).astype(bf16)

    in_maps = []
    for c in range(8):
        b, g = c // 4, c % 4
        sl = slice(g * DQ, (g + 1) * DQ)
        in_maps.append(
            {
                "xqT": xT[("q", b)],
                "xkT": xT[("k", b)],
                "xvT": xT[("v", b)],
                "wq": np.ascontiguousarray(Wq[:, sl]).astype(bf16),
                "wk": np.ascontiguousarray(Wk[:, sl]).astype(bf16),
                "wv": np.ascontiguousarray(Wv[:, sl]).astype(bf16),
                "wo": np.ascontiguousarray(Wo[sl, :]).astype(bf16),
                "bq": np.ascontiguousarray(bq[sl]),
                "bk": np.ascontiguousarray(bk[sl]),
                "bv": np.ascontiguousarray(bv[sl]),
            }
        )

    nc = _build()
    res = run_bass_kernel_spmd(
        nc, in_maps, core_ids=list(range(8)), trace=TRACE
    )
    LAST_RESULTS = res

    full = np.zeros((2, S, D), np.float32)
    for c in range(8):
        full[c // 4] += res.results[c]["out"]
    full += bo[None, None, :]
    return full.reshape(2, S, 1, D)


# revision 19
# speedup vs baseline: 1.0467x; 1.0004x over previous
"""MultiHeadAttention Trainium2 kernel (8 NeuronCores, SPMD).

Sharding: data-parallel over batch (B=2), tensor-parallel over heads
(16 heads -> 4 per core).  Core c handles batch b=c//4, head group
g=c%4 (heads 4g..4g+3).  Wq/Wk/Wv are split column-wise, Wo row-wise;
the per-core Wo partial outputs are summed on the host (replaces the
all-reduce).

Device dataflow per core (bf16 matmuls, f32 PSUM accumulation):
  qT = Wq_g^T x^T   [256, 2048]   (heads on partitions, dh=64 each)
  kT = Wk_g^T x^T   [256, 2048]
  v  = x Wv_g       [2048, 256] stored interleaved with a ones column
                    per head: vaug[st] = [vA|1|vB|1|vC|1|vD|1]
  per (s_q chunk of 512, head pair):
    logitsT[s_k, s_q] = kT^T qT / 8       (two heads packed in PE row
                                           groups, K=64 each)
    p = exp(logitsT)  on ScalarE, scale=1/8 fused, bf16 out
    accT[65, s_q] += vaug_h^T p           (row 64 = softmax denominator)
    outcatT[h] = accT[0:64] * bcast(1/accT[64])   (deferred softmax norm)
  partial = outcatT^T Wo_g  -> DRAM f32
"""

import sys

import numpy as np

sys.path.insert(0, "/opt/trn_rl_repo")

import ml_dtypes  # noqa: E402

import concourse.bass as bass  # noqa: E402
import concourse.mybir as mybir  # noqa: E402
import concourse.tile as tile  # noqa: E402
from concourse import bacc  # noqa: E402
from concourse.bass import ts  # noqa: E402
from concourse.bass_utils import run_bass_kernel_spmd  # noqa: E402

S = 2048  # sequence length (S * X)
D = 1024  # model dim
H = 16  # total heads
HL = 4  # heads per core
DH = 64  # head dim
DQ = HL * DH  # per-core projection width = 256
NK = D // 128  # K tiles for projections = 8
NST = S // 128  # s_k tiles = 16
NCH = S // 512  # s_q chunks = 4

BF16 = mybir.dt.bfloat16
F32 = mybir.dt.float32

TRACE = False
LAST_RESULTS = None

_BUILT = None


def _emit(ctx, tc, io):
    nc = tc.nc
    xq, xk, xv = io["xqT"], io["xkT"], io["xvT"]
    wq, wk, wv, wo = io["wq"], io["wk"], io["wv"], io["wo"]
    bq, bk, bv = io["bq"], io["bk"], io["bv"]
    out = io["out"]

    consts = ctx.enter_context(tc.tile_pool(name="consts", bufs=1))
    xin = ctx.enter_context(tc.tile_pool(name="xin", bufs=1))
    qk = ctx.enter_context(tc.tile_pool(name="qk", bufs=1))
    ptiles = ctx.enter_context(tc.tile_pool(name="ptiles", bufs=4))
    norm = ctx.enter_context(tc.tile_pool(name="norm", bufs=3))
    osb_pool = ctx.enter_context(tc.tile_pool(name="osb", bufs=3))
    psum_mm = ctx.enter_context(tc.tile_pool(name="psum_mm", bufs=4, space="PSUM"))
    psum_lg = ctx.enter_context(tc.tile_pool(name="psum_lg", bufs=2, space="PSUM"))

    wq_t = [consts.tile([128, DQ], BF16, tag=f"wq{k}", name=f"wq{k}") for k in range(NK)]
    wk_t = [consts.tile([128, DQ], BF16, tag=f"wk{k}", name=f"wk{k}") for k in range(NK)]
    wv_t = [consts.tile([128, DQ], BF16, tag=f"wv{k}", name=f"wv{k}") for k in range(NK)]
    wo_t = [consts.tile([128, D], BF16, tag=f"wo{k}", name=f"wo{k}") for k in range(2)]
    xq_t = [xin.tile([128, S], BF16, tag=f"xq{k}", name=f"xq{k}") for k in range(NK)]
    xk_t = [xin.tile([128, S], BF16, tag=f"xk{k}", name=f"xk{k}") for k in range(NK)]
    xv_t = [xin.tile([128, S], BF16, tag=f"xv{k}", name=f"xv{k}") for k in range(NK)]
    # bq/bk as [128, 2] per-partition scalars (col j = dq 128j..128j+127)
    bq_sb = consts.tile([128, 2], F32, tag="bq", name="bq_sb")
    bk_sb = consts.tile([128, 2], F32, tag="bk", name="bk_sb")
    bv_sb = consts.tile([128, DQ], F32, tag="bv", name="bv_sb")

    # PE warmup: ~10us of dummy back-to-back matmuls at t=0, while the PE
    # would otherwise sit idle waiting for input DMA.  The PE clock gate
    # (HAM) defaults to 4/8 throttle (1.2 GHz) and only releases after
    # ~3.4us of sustained activity; without this the whole DMA-paced
    # front (k/v/q projections) runs at half clock.
    wu_sb = consts.tile([128, 512], BF16, tag="wu", name="wu_sb")
    nc.vector.memset(wu_sb[:], 1.0)
    wu_ps = psum_lg.tile([128, 512], F32, tag="lg", name="lg")
    for _ in range(32):
        nc.tensor.matmul(wu_ps[:], wu_sb[:, 0:128], wu_sb[:], start=True, stop=True)

    # Tiny bias DMAs on the gpsimd (SWDGE) queue; all bulk traffic on the
    # sync (HWDGE) queue in dependency order (k-projection inputs first,
    # then v, then q, Wo last).
    nc.gpsimd.dma_start(
        out=bk_sb[:], in_=bass.AP(tensor=bk.tensor, offset=bk.offset, ap=[[1, 128], [128, 2]])
    )
    nc.gpsimd.dma_start(
        out=bq_sb[:], in_=bass.AP(tensor=bq.tensor, offset=bq.offset, ap=[[1, 128], [128, 2]])
    )
    nc.gpsimd.dma_start(
        out=bv_sb[:], in_=bass.AP(tensor=bv.tensor, offset=bv.offset, ap=[[0, 128], [1, DQ]])
    )
    for k in range(NK):
        nc.sync.dma_start(wk_t[k][:], wk[ts(k, 128), :])
    for k in range(NK):
        nc.sync.dma_start(xk_t[k][:], xk[ts(k, 128), :])
    for k in range(NK):
        nc.sync.dma_start(wv_t[k][:], wv[ts(k, 128), :])
    for k in range(NK):
        nc.sync.dma_start(xv_t[k][:], xv[ts(k, 128), :])
    for k in range(NK):
        nc.sync.dma_start(wq_t[k][:], wq[ts(k, 128), :])
    for k in range(NK):
        nc.sync.dma_start(xq_t[k][:], xq[ts(k, 128), :])
    for k in range(2):
        nc.sync.dma_start(wo_t[k][:], wo[ts(k, 128), :])

    # ---- projections: qT, kT = [256, 2048] as 2 tiles of [128, 2048] ----
    qT = [qk.tile([128, S], BF16, tag=f"qT{m}", name=f"qT{m}") for m in range(2)]
    kT = [qk.tile([128, S], BF16, tag=f"kT{m}", name=f"kT{m}") for m in range(2)]

    def qk_group(w_t, x_t, dst, b_sb, m, c, on_act):
        """One PSUM accumulation group of a q/k projection (+bias, ->bf16)."""
        ps = psum_mm.tile([128, 512], F32, tag="mm", name="mm")
        for k in range(NK):
            nc.tensor.matmul(
                ps[:],
                w_t[k][:, ts(m, 128)],
                x_t[k][:, ts(c, 512)],
                start=(k == 0),
                stop=(k == NK - 1),
            )
        if on_act:  # prefix phase: ScalarE is idle there
            nc.scalar.add(dst[m][:, ts(c, 512)], ps[:], b_sb[:, m : m + 1])
        else:  # injected into attention: keep ScalarE free for exp
            nc.vector.tensor_scalar_add(dst[m][:, ts(c, 512)], ps[:], b_sb[:, m : m + 1])

    # vaug[st] = [vA|1|vB|1|vC|1|vD|1]  [128, 260]
    vaug = [qk.tile([128, HL * (DH + 1)], BF16, tag=f"vaug{st}", name=f"vaug{st}") for st in range(NST)]

    def v_block(blk):
        # 4 st tiles per block, k outermost: each k-pass runs as soon as
        # xv tile k lands, so v-projection paces with its DMA instead of
        # serializing after it.
        sts = range(blk * 4, blk * 4 + 4)
        pss = {st: psum_mm.tile([128, DQ], F32, tag="mm", name="mm") for st in sts}
        for k in range(NK):
            for st in sts:
                nc.tensor.matmul(
                    pss[st][:],
                    xv_t[k][:, ts(st, 128)],
                    wv_t[k][:],
                    start=(k == 0),
                    stop=(k == NK - 1),
                )
        for st in sts:
            for h in range(HL):
                nc.vector.tensor_add(
                    vaug[st][:, h * 65 : h * 65 + 64],
                    pss[st][:, ts(h, DH)],
                    bv_sb[:, ts(h, DH)],
                )
                nc.vector.memset(vaug[st][:, h * 65 + 64 : h * 65 + 65], 1.0)

    octT = [qk.tile([128, S], BF16, tag=f"octT{m}", name=f"octT{m}") for m in range(2)]

    def wo_group(c, smt, ncho, on_act=False):
        row = c * 512 + smt * 128
        ps = psum_mm.tile([128, 512], F32, tag="mm", name="mm")
        for k in range(2):
            nc.tensor.matmul(
                ps[:],
                octT[k][:, row : row + 128],
                wo_t[k][:, ts(ncho, 512)],
                start=(k == 0),
                stop=(k == 1),
            )
        osb = osb_pool.tile([128, 512], F32, tag="osb", name="osb")
        if on_act:  # tail drain: ScalarE is idle after the last exp
            nc.scalar.copy(osb[:], ps[:])
        else:
            nc.vector.tensor_copy(osb[:], ps[:])
        # alternate store queues so the tail's 2.1MB of output DMA drains
        # on two engines instead of one
        dma = nc.gpsimd if (smt + ncho) % 2 else nc.sync
        dma.dma_start(out[row : row + 128, ts(ncho, 512)], osb[:])

    # ---- prefix: everything attention chunk 0 needs: all of kT and vaug,
    # qT chunk 0.  qT chunks 1-3 and all Wo groups are deferred into
    # pe_queue and drained inside the attention pipeline, hiding under
    # ScalarE exp time.
    for m in range(2):
        for c in range(NCH):
            qk_group(wk_t, xk_t, kT, bk_sb, m, c, on_act=True)
    for blk in range(4):
        v_block(blk)
    for m in range(2):
        qk_group(wq_t, xq_t, qT, bq_sb, m, 0, on_act=True)

    # ---- attention: one flat software pipeline over all (c, pr, st)
    # steps.  The logits+exp front stream runs LAG steps ahead of the
    # accumulation stream, including across pair/chunk boundaries, so the
    # ScalarE exp stream never waits for a pipeline refill.
    steps = [(c, pr, st) for c in range(NCH) for pr in range(2) for st in range(NST)]
    LAG = 2

    def qg(m, c):
        return lambda: qk_group(wq_t, xq_t, qT, bq_sb, m, c, on_act=False)

    pe_queue = []
    acc_map = {}
    p_map = {}

    def emit_front(c, pr, st):
        lg = psum_lg.tile([128, 1024], F32, tag="lg", name="lg")
        for hh in range(2):
            nc.tensor.matmul(
                lg[:, ts(hh, 512)],
                kT[pr][ts(hh, 64), ts(st, 128)],
                qT[pr][ts(hh, 64), ts(c, 512)],
                start=True,
                stop=True,
            )
        p = ptiles.tile([128, 1024], BF16, tag="p", name="p")
        nc.scalar.activation(p[:], lg[:], mybir.ActivationFunctionType.Exp, scale=0.125)
        p_map[(c, pr, st)] = p

    def emit_acc(c, pr, st):
        if st == 0:
            acc_map[(c, pr)] = [
                psum_mm.tile([65, 512], F32, tag="mm", name="mm") for _ in range(2)
            ]
        acc = acc_map[(c, pr)]
        pp = p_map.pop((c, pr, st))
        for hh in range(2):
            h = 2 * pr + hh
            nc.tensor.matmul(
                acc[hh][:],
                vaug[st][:, h * 65 : h * 65 + 65],
                pp[:, ts(hh, 512)],
                start=(st == 0),
                stop=(st == NST - 1),
            )
        if st == NST - 1:
            # normalize: octT[pr][64*hh, chunk c] = acc[0:64] / acc[64].
            # Fast copies release the PSUM accumulators; broadcast +
            # approx-reciprocal + mul run off the critical path.
            for hh in range(2):
                un = norm.tile([64, 512], BF16, tag="un", name="un")
                nc.vector.tensor_copy(un[:], acc[hh][0:64, :])
                den = norm.tile([1, 512], F32, tag="den", name="den")
                nc.vector.tensor_copy(den[:], acc[hh][64:65, :])
                bc = norm.tile([64, 512], F32, tag="bcast", name="bcast")
                nc.gpsimd.partition_broadcast(bc[:], den[:])
                rbc = norm.tile([64, 512], F32, tag="rbc", name="rbc")
                nc.vector.reciprocal_approx_fast(rbc[:], bc[:])
                nc.vector.tensor_mul(octT[pr][ts(hh, 64), ts(c, 512)], un[:], rbc[:])
            del acc_map[(c, pr)]
            if pr == 1:
                kwargs = {"on_act": True} if c == NCH - 1 else {}
                pe_queue.extend(
                    (lambda cc=c, smt=smt, ncho=ncho, kw=kwargs: wo_group(cc, smt, ncho, **kw))
                    for smt in range(4)
                    for ncho in range(2)
                )

    for i, s in enumerate(steps):
        c, pr, st = s
        if pr == 0 and st == 0 and c + 1 < NCH:
            pe_queue.extend(qg(m, c + 1) for m in range(2))
        emit_front(c, pr, st)
        if i >= LAG:
            emit_acc(*steps[i - LAG])
        # inject deferred work at odd steps, but keep the first steps of
        # each chunk clean (exp-stream gaps cluster at chunk boundaries)
        if st % 2 == 1 and not (pr == 0 and st == 1) and pe_queue:
            pe_queue.pop(0)()

    for i in range(len(steps) - LAG, len(steps)):
        emit_acc(*steps[i])

    for g in pe_queue:
        g()


def _build():
    global _BUILT
    if _BUILT is not None:
        return _BUILT
    nc = bacc.Bacc(
        "TRN2",
        target_bir_lowering=False,
        debug=False,
        enable_asserts=False,
        num_devices=8,
    )
    io = {}
    io["xqT"] = nc.dram_tensor("xqT", [D, S], BF16, kind="ExternalInput").ap()
    io["xkT"] = nc.dram_tensor("xkT", [D, S], BF16, kind="ExternalInput").ap()
    io["xvT"] = nc.dram_tensor("xvT", [D, S], BF16, kind="ExternalInput").ap()
    io["wq"] = nc.dram_tensor("wq", [D, DQ], BF16, kind="ExternalInput").ap()
    io["wk"] = nc.dram_tensor("wk", [D, DQ], BF16, kind="ExternalInput").ap()
    io["wv"] = nc.dram_tensor("wv", [D, DQ], BF16, kind="ExternalInput").ap()
    io["wo"] = nc.dram_tensor("wo", [DQ, D], BF16, kind="ExternalInput").ap()
    io["bq"] = nc.dram_tensor("bq", [DQ], F32, kind="ExternalInput").ap()
    io["bk"] = nc.dram_tensor("bk", [DQ], F32, kind="ExternalInput").ap()
    io["bv"] = nc.dram_tensor("bv", [DQ], F32, kind="ExternalInput").ap()
    io["out"] = nc.dram_tensor("out", [S, D], F32, kind="ExternalOutput").ap()
    from contextlib import ExitStack

    with tile.TileContext(nc) as tc, ExitStack() as ctx:
        _emit(ctx, tc, io)
    nc.compile()
    _BUILT = nc
    return nc


def kernel(**inputs):
    global LAST_RESULTS
    bf16 = ml_dtypes.bfloat16
    query = np.asarray(inputs["query"], np.float32).reshape(2, S, D)
    key = np.asarray(inputs["key"], np.float32).reshape(2, S, D)
    value = np.asarray(inputs["value"], np.float32).reshape(2, S, D)
    Wq = np.asarray(inputs["Wq"], np.float32)
    Wk = np.asarray(inputs["Wk"], np.float32)
    Wv = np.asarray(inputs["Wv"], np.float32)
    Wo = np.asarray(inputs["Wo"], np.float32)
    bq = np.asarray(inputs["bq"], np.float32)
    bk = np.asarray(inputs["bk"], np.float32)
    bv = np.asarray(inputs["bv"], np.float32)
    bo = np.asarray(inputs["bo"], np.float32)

    xT = {}
    for b in range(2):
        xT[("q", b)] = np.ascontiguousarray(query[b].T).astype(bf16)
        xT[("k", b)] = np.ascontiguousarray(key[b].T).astype(bf16)
        xT[("v", b)] = np.ascontiguousarray(value[b].T).astype(bf16)

    in_maps = []
    for c in range(8):
        b, g = c // 4, c % 4
        sl = slice(g * DQ, (g + 1) * DQ)
        in_maps.append(
            {
                "xqT": xT[("q", b)],
                "xkT": xT[("k", b)],
                "xvT": xT[("v", b)],
                "wq": np.ascontiguousarray(Wq[:, sl]).astype(bf16),
                "wk": np.ascontiguousarray(Wk[:, sl]).astype(bf16),
                "wv": np.ascontiguousarray(Wv[:, sl]).astype(bf16),
                "wo": np.ascontiguousarray(Wo[sl, :]).astype(bf16),
                "bq": np.ascontiguousarray(bq[sl]),
                "bk": np.ascontiguousarray(bk[sl]),
                "bv": np.ascontiguousarray(bv[sl]),
            }
        )

    nc = _build()
    res = run_bass_kernel_spmd(
        nc, in_maps, core_ids=list(range(8)), trace=TRACE
    )
    LAST_RESULTS = res

    full = np.zeros((2, S, D), np.float32)
    for c in range(8):
        full[c // 4] += res.results[c]["out"]
    full += bo[None, None, :]
    return full.reshape(2, S, 1, D)
